# revision 12
# baseline (speedup 1.0000x reference)
"""Trainium2 Bass kernel for nn_Intersection (product mode).

Math: out = relu(a @ feats.T) @ relu(b @ feats.T).T
  a [1024, 2048], b [1024, 2048], feats [128, 2048] -> out [1024, 1024]

Sharding: 4x2 grid over 8 cores. Core (gi, gj) computes the output block
rows [gi*256, (gi+1)*256) x cols [gj*512, (gj+1)*512) from a-rows gi,
b-rows gj, feats replicated. This needs 7 MB of HBM reads per core vs
10 MB for the 8x1 data-parallel split.

Layout trick: the PE contracts over the partition dim, so both GEMM
operands must be k-major. We pre-transpose on the host and pack
[featsT | aT_slice | bT_slice] into one [2048, 896] array per core so
each k-chunk is ONE contiguous 448KB DMA. Stage 1 computes a_fk.T and
b_fk.T directly in the j-major layout stage 2 consumes -> no on-chip
transposes anywhere.
"""

import os
import sys

import numpy as np

if "/opt/trn_rl_repo" not in sys.path:
    sys.path.insert(0, "/opt/trn_rl_repo")

# Grid: 4 row-groups of a (256 rows), 2 col-groups of b (512 rows)
GI, GJ = 4, 2
MA, NB = 256, 512  # per-core a-rows / b-rows
K = 2048
F = 128  # feats rows
KC = K // 128  # 16 k-chunks
XW = F + MA + NB  # 896 packed columns

_NC_CACHE = {}


def _build_nc(dt_name: str, reps: int = 1, loop_reps: int = 1):
    from contextlib import nullcontext

    import concourse.mybir as mybir
    import concourse.tile as tile
    from concourse import bacc

    f32 = mybir.dt.float32
    dt_mm = getattr(mybir.dt, dt_name)

    nc = bacc.Bacc(None, target_bir_lowering=False, debug=False)
    x = nc.dram_tensor("x", [K, XW], dt_mm, kind="ExternalInput")
    o = nc.dram_tensor("o", [MA, NB], f32, kind="ExternalOutput")

    with tile.TileContext(nc) as tc:
      with (
          tc.For_i(0, loop_reps, 1) if loop_reps > 1 else nullcontext()
      ):
        with (
            tc.tile_pool(name="xin", bufs=6) as xin,
            tc.tile_pool(name="work", bufs=2) as work,
            tc.tile_pool(name="psum", bufs=1, space="PSUM") as psum,
        ):
          for _rep in range(reps):
            # Stage 1: accumulate a_fk.T [j=128, m=256] and b_fk.T
            # [j=128, n=512] over 16 k-chunks. featsT chunk (cols 0:128)
            # is the stationary operand shared by both matmuls.
            bsplit = os.environ.get("KERNEL_BSPLIT", "0") == "1"
            NH = NB // 2  # b-side half width (256)
            psum_a = psum.tile([128, MA], f32, name="psum_a")
            if bsplit:
                psum_b = [
                    psum.tile([128, NH], f32, name=f"psum_b{h}") for h in range(2)
                ]
            else:
                psum_bf = psum.tile([128, NB], f32, name="psum_bf")
                psum_b = [psum_bf[:, 0:NH], psum_bf[:, NH:NB]]
            for c in range(KC):
                xt = xin.tile([128, XW], dt_mm, name="xt")
                # Alternate the two HWDGE rings (sync / scalar) so the DMA
                # streams issue in parallel instead of serializing on SP.
                dma_eng = nc.sync if c % 2 == 0 else nc.scalar
                if c == 0:
                    # Split chunk 0 so the first matmul's operands (feats +
                    # a-slice, cols 0:384) land ~0.7us earlier.
                    nc.sync.dma_start(xt[:, 0 : F + MA], x[0:128, 0 : F + MA])
                    nc.scalar.dma_start(xt[:, F + MA : XW], x[0:128, F + MA : XW])
                else:
                    dma_eng.dma_start(xt[:], x[c * 128 : (c + 1) * 128, :])
                # a first so psum_a finishes earliest, then the two b halves.
                nc.tensor.matmul(
                    psum_a[:],
                    xt[:, 0:F],
                    xt[:, F : F + MA],
                    start=(c == 0),
                    stop=(c == KC - 1),
                )
                if bsplit:
                    for h in range(2):
                        nc.tensor.matmul(
                            psum_b[h][:],
                            xt[:, 0:F],
                            xt[:, F + MA + h * NH : F + MA + (h + 1) * NH],
                            start=(c == 0),
                            stop=(c == KC - 1),
                        )
                else:
                    nc.tensor.matmul(
                        psum_bf[:],
                        xt[:, 0:F],
                        xt[:, F + MA : XW],
                        start=(c == 0),
                        stop=(c == KC - 1),
                    )

            # Relus, spread across ACT and DVE so they pipeline with the
            # last matmuls and with each other.
            ra = work.tile([128, MA], dt_mm, name="ra")
            rbf = work.tile([128, NB], dt_mm, name="rbf")
            rb = [rbf[:, 0:NH], rbf[:, NH:NB]]
            nc.scalar.activation(ra[:], psum_a[:], mybir.ActivationFunctionType.Relu)
            nc.vector.tensor_scalar_max(rb[0][:], psum_b[0][:], 0.0)
            nc.scalar.activation(rb[1][:], psum_b[1][:], mybir.ActivationFunctionType.Relu)

            # Stage 2: out[m, n] = sum_j ra[j, m] * rb[j, n], j = 128.
            # 2 m-subtiles x 2 n-halves, copies alternate ACT/DVE, output
            # DMAs alternate the two HWDGE rings.
            copy_engs = [nc.scalar.copy, nc.vector.tensor_copy]
            out_dma_engs = [nc.sync, nc.scalar]
            s2split = os.environ.get("KERNEL_S2SPLIT", "1") == "1"
            k = 0
            for sub in range(MA // 128):
                ot = work.tile([128, NB], f32, name=f"ot{sub}")
                if s2split:
                    for h in range(2):
                        po = psum.tile([128, NH], f32, name=f"po{sub}{h}")
                        nc.tensor.matmul(
                            po[:],
                            ra[:, sub * 128 : (sub + 1) * 128],
                            rb[h][:],
                            start=True,
                            stop=True,
                        )
                        copy_engs[k % 2](ot[:, h * NH : (h + 1) * NH], po[:])
                        k += 1
                else:
                    po = psum.tile([128, NB], f32, name=f"po{sub}")
                    nc.tensor.matmul(
                        po[:],
                        ra[:, sub * 128 : (sub + 1) * 128],
                        rbf[:],
                        start=True,
                        stop=True,
                    )
                    # split the PSUM->SBUF copy across ACT and DVE
                    for h in range(2):
                        copy_engs[k % 2](ot[:, h * NH : (h + 1) * NH], po[:, h * NH : (h + 1) * NH])
                        k += 1
                for h in range(2):
                    out_dma_engs[h].dma_start(
                        o[sub * 128 : (sub + 1) * 128, h * NH : (h + 1) * NH],
                        ot[:, h * NH : (h + 1) * NH],
                    )

    nc.compile()
    return nc


def _get_nc():
    dt_name = os.environ.get("KERNEL_MM_DT", "float32")
    reps = int(os.environ.get("KERNEL_REPS", "1"))
    loop_reps = int(os.environ.get("KERNEL_LOOP_REPS", "1"))
    key = (dt_name, reps, loop_reps)
    if key not in _NC_CACHE:
        _NC_CACHE[key] = _build_nc(dt_name, reps, loop_reps)
    return _NC_CACHE[key]


def _make_in_maps(a, b, feats):
    a = np.asarray(a, dtype=np.float32)
    b = np.asarray(b, dtype=np.float32)
    feats = np.asarray(feats, dtype=np.float32)
    aT = np.ascontiguousarray(a.T)  # [2048, 1024]
    bT = np.ascontiguousarray(b.T)  # [2048, 1024]
    fT = np.ascontiguousarray(feats.T)  # [2048, 128]
    in_maps = []
    for gi in range(GI):
        for gj in range(GJ):
            X = np.empty((K, XW), np.float32)
            X[:, :F] = fT
            X[:, F : F + MA] = aT[:, gi * MA : (gi + 1) * MA]
            X[:, F + MA :] = bT[:, gj * NB : (gj + 1) * NB]
            in_maps.append({"x": X})
    return in_maps


def _assemble(results):
    out = np.empty((GI * MA, GJ * NB), np.float32)
    for gi in range(GI):
        for gj in range(GJ):
            out[gi * MA : (gi + 1) * MA, gj * NB : (gj + 1) * NB] = results[
                gi * GJ + gj
            ]["o"]
    return out


def run(a, b, feats, trace=False, **spmd_kwargs):
    from concourse.bass_utils import run_bass_kernel_spmd

    nc = _get_nc()
    in_maps = _make_in_maps(a, b, feats)
    res = run_bass_kernel_spmd(
        nc, in_maps, core_ids=list(range(GI * GJ)), trace=trace, **spmd_kwargs
    )
    return _assemble(res.results), res


def kernel(a, b, feats):
    out, _ = run(a, b, feats, trace=False)
    return out
